# revision 11
# baseline (speedup 1.0000x reference)
"""Trainium2 Bass kernel for NeighborhoodReasoner (gnn_message_passing).

Computation (per batch b):
    neighbors[n,k,:] = X[knn[n,k], :]            # gather
    m = neighbors.mean(k)                        # [N, D]
    feats = [x, m, x-m]                          # [N, 3D]
    out = x + (gelu(feats @ W1 + b1) @ W2 + b2)

Identity used to fold the concat:  feats@W1 = x@(W1a+W1c) + m@(W1b-W1c)
where W1 = [W1a; W1b; W1c] stacked along the 3D input dim.

Sharding: 8 cores = 4 batches x 2 node-halves. Each core gathers from the
full per-batch node table in HBM (bf16 copy, host-cast) via SWDGE
dma_gather, reduces the K=16 neighbor sum on the tensor engine
(gathered-chunk-as-stationary matmul against identity, which emits the
transposed layout the MLP matmuls need), applies the MLP, and streams its
half of the output back.  Residual path and x^T stay fp32; only the
neighbor-mean path and MLP weights ride bf16 (error ~1e-3 of absmax).
"""

import sys

import numpy as np

if "/opt/trn_rl_repo" not in sys.path:
    sys.path.insert(0, "/opt/trn_rl_repo")

B, N, K, D = 4, 20000, 16, 128
N_CORES = 8
HALF = N // 2          # nodes per core
TILE = 512             # nodes per dma_gather call
PAD = 10240            # HALF rounded up to a multiple of TILE
TILES = PAD // TILE
CHUNKS = TILE * K // 128   # gather chunks per call (64)
INNER = TILE // 128        # compute subtiles per call (4)

_NC_CACHE = {}


def _build_nc():
    import concourse.mybir as mybir
    import concourse.tile as tile
    from concourse import bacc
    from concourse._compat import get_trn_type

    f32 = mybir.dt.float32
    bf16 = mybir.dt.bfloat16
    AF = mybir.ActivationFunctionType

    nc = bacc.Bacc(
        get_trn_type() or "TRN2",
        target_bir_lowering=False,
        debug=False,
        num_swdge_queues=2,
        dynamic_dma_scratch_size=32768,
    )
    x_table = nc.dram_tensor("x_table", [N, D], bf16, kind="ExternalInput")
    x_own = nc.dram_tensor("x_own", [PAD, D], f32, kind="ExternalInput")
    idx = nc.dram_tensor("idx", [128, PAD], mybir.dt.int16, kind="ExternalInput")
    w1 = nc.dram_tensor("w1", [3 * D, D], f32, kind="ExternalInput")
    b1 = nc.dram_tensor("b1", [D], f32, kind="ExternalInput")
    w2 = nc.dram_tensor("w2", [D, D], f32, kind="ExternalInput")
    b2 = nc.dram_tensor("b2", [D], f32, kind="ExternalInput")
    ident = nc.dram_tensor("ident", [D, D], f32, kind="ExternalInput")
    out = nc.dram_tensor("out", [PAD, D], f32, kind="ExternalOutput")

    with tile.TileContext(nc) as tc:
        with (
            tc.tile_pool(name="singles", bufs=1) as singles,
            tc.tile_pool(name="gpool", bufs=4) as gpool,
            tc.tile_pool(name="xpool", bufs=4) as xpool,
            tc.tile_pool(name="opool", bufs=4) as opool,
            tc.tile_pool(name="spool", bufs=3) as spool,
            tc.tile_pool(name="psum", bufs=1, space="PSUM") as psum,
        ):
            ident_sb = singles.tile([D, D], f32)
            nc.sync.dma_start(out=ident_sb, in_=ident[:, :])
            identb_sb = singles.tile([D, D], bf16)
            nc.vector.tensor_copy(identb_sb, ident_sb)
            w1_sb = singles.tile([128, 3, D], f32)
            nc.sync.dma_start(out=w1_sb, in_=w1.rearrange("(t p) h -> p t h", p=128))
            a_sb = singles.tile([128, D], bf16)
            c_sb = singles.tile([128, D], bf16)
            nc.vector.tensor_add(a_sb, w1_sb[:, 0, :], w1_sb[:, 2, :])
            nc.vector.tensor_tensor(
                c_sb, w1_sb[:, 1, :], w1_sb[:, 2, :], mybir.AluOpType.subtract
            )
            w2f_sb = singles.tile([D, D], f32)
            nc.sync.dma_start(out=w2f_sb, in_=w2[:, :])
            w2_sb = singles.tile([D, D], bf16)
            nc.vector.tensor_copy(w2_sb, w2f_sb)
            b1_sb = singles.tile([D, 1], f32)
            nc.sync.dma_start(out=b1_sb, in_=b1.rearrange("(p o) -> p o", o=1))
            b2_sb = singles.tile([D, 1], f32)
            nc.sync.dma_start(out=b2_sb, in_=b2.rearrange("(p o) -> p o", o=1))
            idx_sb = singles.tile([128, PAD], mybir.dt.int16)
            nc.sync.dma_start(out=idx_sb, in_=idx[:, :])

            for t in range(TILES):
                g = gpool.tile([128, CHUNKS, D], bf16, tag="g")
                nc.gpsimd.dma_gather(
                    out_ap=g[:],
                    in_ap=x_table[:, :],
                    idxs_ap=idx_sb[:, t * TILE : (t + 1) * TILE],
                    num_idxs=TILE * K,
                    num_idxs_reg=TILE * K,
                    elem_size=D,
                    single_packet=False,
                    queue_num=t % 2,
                )
                xt = xpool.tile([128, INNER, D], f32, tag="x")
                nc.sync.dma_start(
                    out=xt,
                    in_=x_own[t * TILE : (t + 1) * TILE, :].rearrange(
                        "(s p) d -> p s d", p=128
                    ),
                )
                ot = opool.tile([128, INNER, D], f32, tag="o")
                for i in range(INNER):
                    # Neighbor sum, transposed: psum[f, p] = sum_c g[p, 16i+c, f]
                    p_mt = psum.tile([D, 128], f32, tag="mt")
                    for c in range(K):
                        nc.tensor.matmul(
                            p_mt,
                            lhsT=g[:, K * i + c, :],
                            rhs=identb_sb,
                            start=(c == 0),
                            stop=(c == K - 1),
                        )
                    mt_sb = spool.tile([D, 128], bf16, tag="mt_sb")
                    nc.scalar.activation(
                        mt_sb, p_mt, AF.Copy, bias=0.0, scale=1.0 / K
                    )
                    # x^T (fp32 stationary, fp32 identity)
                    p_xt = psum.tile([D, 128], f32, tag="xt")
                    nc.tensor.matmul(
                        p_xt, lhsT=xt[:, i, :], rhs=ident_sb, start=True, stop=True
                    )
                    xt_sb = spool.tile([D, 128], bf16, tag="xt_sb")
                    nc.scalar.copy(xt_sb, p_xt)
                    # pre^T = A^T x^T + C^T m^T   (all bf16 in, fp32 psum)
                    p_pre = psum.tile([D, 128], f32, tag="pre")
                    nc.tensor.matmul(p_pre, lhsT=a_sb, rhs=xt_sb, start=True, stop=False)
                    nc.tensor.matmul(p_pre, lhsT=c_sb, rhs=mt_sb, start=False, stop=True)
                    h_sb = spool.tile([D, 128], bf16, tag="h")
                    nc.scalar.activation(h_sb, p_pre, AF.Gelu, bias=b1_sb, scale=1.0)
                    # upd^T = W2^T h^T (+ b2 on the copy back)
                    p_updt = psum.tile([D, 128], f32, tag="updt")
                    nc.tensor.matmul(p_updt, lhsT=w2_sb, rhs=h_sb, start=True, stop=True)
                    updt_sb = spool.tile([D, 128], bf16, tag="updt_sb")
                    nc.scalar.activation(
                        updt_sb, p_updt, AF.Identity, bias=b2_sb, scale=1.0
                    )
                    # back to node-major and residual-add (x stays fp32)
                    p_upd = psum.tile([128, D], f32, tag="upd")
                    nc.tensor.matmul(
                        p_upd, lhsT=updt_sb, rhs=identb_sb, start=True, stop=True
                    )
                    nc.vector.tensor_add(ot[:, i, :], xt[:, i, :], p_upd)
                nc.sync.dma_start(
                    out=out[t * TILE : (t + 1) * TILE, :].rearrange(
                        "(s p) d -> p s d", p=128
                    ),
                    in_=ot,
                )
    nc.compile()
    return nc


def get_nc():
    if "nc" not in _NC_CACHE:
        _NC_CACHE["nc"] = _build_nc()
    return _NC_CACHE["nc"]


def _marshal_core(core, node_embeddings, knn_indices, W1, b1, W2, b2):
    import ml_dtypes

    b, h = core // 2, core % 2
    n0 = h * HALF
    x_own = np.zeros((PAD, D), np.float32)
    x_own[:HALF] = node_embeddings[b, n0 : n0 + HALF]
    loc = np.zeros((PAD, K), np.int64)
    loc[:HALF] = knn_indices[b, n0 : n0 + HALF]
    # slot j within gather call t: chunk c = j // 128, partition p = j % 128
    # maps to (node = t*512 + (c//16)*128 + p, k = c % 16)
    L = loc.reshape(TILES, INNER, 128, K)            # [t, i, p, k]
    flat = L.transpose(0, 1, 3, 2).reshape(TILES, TILE * K)
    # wrapped 16-partition layout: wrapped[p16, col] = flat[col*16 + p16]
    wr = flat.reshape(TILES, TILE, 16).transpose(0, 2, 1)   # [t, 16, TILE]
    rep = np.tile(wr, (1, 8, 1))                     # replicated for 8 Q7 cores
    idx = np.ascontiguousarray(
        rep.transpose(1, 0, 2).reshape(128, PAD)
    ).astype(np.int16)
    return {
        "x_table": node_embeddings[b].astype(ml_dtypes.bfloat16),
        "x_own": x_own,
        "idx": idx,
        "w1": np.ascontiguousarray(W1),
        "b1": np.ascontiguousarray(b1),
        "w2": np.ascontiguousarray(W2),
        "b2": np.ascontiguousarray(b2),
        "ident": np.eye(D, dtype=np.float32),
    }


def make_in_maps(node_embeddings, knn_indices, W1, b1, W2, b2):
    return [
        _marshal_core(c, node_embeddings, knn_indices, W1, b1, W2, b2)
        for c in range(N_CORES)
    ]


def assemble_output(results):
    out = np.empty((B, N, D), np.float32)
    for c in range(N_CORES):
        b, h = c // 2, c % 2
        out[b, h * HALF : (h + 1) * HALF] = results[c]["out"][:HALF]
    return out


def kernel(
    node_embeddings, knn_indices, W1, b1, W2, b2, _trace=False, _trace_kwargs=None
):
    from concourse import bass_utils

    nc = get_nc()
    in_maps = make_in_maps(node_embeddings, knn_indices, W1, b1, W2, b2)
    res = bass_utils.run_bass_kernel_spmd(
        nc,
        in_maps,
        core_ids=list(range(N_CORES)),
        trace=_trace,
        **(_trace_kwargs or {}),
    )
    out = assemble_output(res.results)
    if _trace:
        return out, res
    return out
